# revision 24
# baseline (speedup 1.0000x reference)
"""AMMLinear (vq_codebook) forward kernel for 8 TRN2 NeuronCores.

Key algebraic fact: the reference's straight-through estimator
    output = real - stop_grad(real - quantized)
is numerically exactly `quantized_output + bias`, so the forward value needs
only:  argmin-distance one-hot  @  fake-quantized lut  + bias.

Distribution: pure data-parallel over the 8192 tokens (1024/core) with ZERO
collectives -- cores run fully independently (no barrier / AllReduce /
AllGather latency, immune to core start skew).  Every core recomputes the
full lut = centroids @ weight on its PE from an fp16 copy of the weight
(single-pass fp16 matmuls, fp32 PSUM accumulation; ~0.4% of q entries shift
by +-1 quantum => ~4e-3 output rel err vs the 2e-2 gate).

The int8 fake-quant scale max|lut|/127 is an x-independent scalar derived
from the weights (offline-precomputable in any real AMM deployment); it is
computed on host and shipped as a per-partition constant, which lets the
quantize fuse into the PSUM-drain: ONE scalar-engine op per lut pair
    u = Identity(lut_psum * (127/max) + 1536) -> fp16
where fp16's ulp in [1024,2048) is exactly 1.0, so the dtype-converting
write rounds RNE to integer, matching jnp.round half-to-even.  The 1536
offset is linear through the one-hot matmul (sum_ck oh = 64 exactly) and is
pre-folded into the epilogue bias as bias - 1536*64*scale.

Scores x.c need fp32-exact argmins (a flipped argmin corrupts a whole
4096-wide output row) but fp32 PE matmuls are 4-5x slower than fp16: x and
the block-diag centroids are split hi/lo into fp16 pairs and scores
accumulate 3 fp16 passes (xh.bh + xh.bl + xl.bh) plus an fp16 c2h/c2l
init-pair in fp32 PSUM -- residual ~2^-22, measured 1 argmin flip in 524288.

Per-core pipeline: lut pairs (PE -> fused quantize on Act) || score tiles
(PE -> argmax chain on DVE -> PE transpose -> one-hot expand via broadcast
DMA + is_equal) -> G: out.T tiles as dense 128-contraction fp16 matmuls
accumulated in PSUM (PE), epilogue Identity(psum*scale + bias') split
across Act/DVE, contiguous DMA out.  G o-tiles are interleaved into the PE
stream as their q chunks and one-hot halves become ready.  Host transposes
the per-core out.T shards (layout only).
"""

import numpy as np

N_TOKENS = 8192
IN_FEAT = 1024
C = 64   # codebooks
KC = 16  # centroids per codebook
S = 16   # subvector length
O = 4096  # out features
NCORES = 8
NLOC = N_TOKENS // NCORES  # 1024 tokens per core
G = 8    # groups of 8 codebooks -> 128-row contraction
TT = NLOC // 128  # 8 token tiles
NCH = 8  # lut o-chunks of 512
OTILES = O // 128  # 32
OFF = 1536.0          # fp16 integer-rounding offset (ulp=1 in [1024,2048))
OFFSUM = 1536.0 * 64  # offset passed through the 64-codebook one-hot sum

_CACHED = {}


def build_nc():
    import concourse.bacc as bacc
    import concourse.mybir as mybir
    import concourse.tile as tile
    from contextlib import ExitStack

    f32 = mybir.dt.float32
    f16 = mybir.dt.float16
    AO = mybir.AluOpType
    AF = mybir.ActivationFunctionType
    X = mybir.AxisListType.X

    nc = bacc.Bacc(
        "TRN2", target_bir_lowering=False, debug=False, num_devices=NCORES
    )

    xh_d = nc.dram_tensor("xh", [128, TT, G, 128], f16, kind="ExternalInput")
    xl_d = nc.dram_tensor("xl", [128, TT, G, 128], f16, kind="ExternalInput")
    w16_d = nc.dram_tensor("w16", [128, NCH, G, 512], f16, kind="ExternalInput")
    bdh_d = nc.dram_tensor("bdh", [128, G, 128], f16, kind="ExternalInput")
    bdl_d = nc.dram_tensor("bdl", [128, G, 128], f16, kind="ExternalInput")
    nc2hl_d = nc.dram_tensor("nc2hl", [2, 1024], f16, kind="ExternalInput")
    or2_d = nc.dram_tensor("or2", [2, 128], f16, kind="ExternalInput")
    biasT2_d = nc.dram_tensor("biasT2", [128, OTILES], f32, kind="ExternalInput")
    inv_d = nc.dram_tensor("inv", [128, 1], f32, kind="ExternalInput")
    scl_d = nc.dram_tensor("scl", [128, 1], f32, kind="ExternalInput")
    kiota_d = nc.dram_tensor("kiota", [128, 1], f16, kind="ExternalInput")
    ioneg_d = nc.dram_tensor("ioneg", [128, 1024], f16, kind="ExternalInput")
    idb_d = nc.dram_tensor("idb", [128, 128], f16, kind="ExternalInput")
    out_d = nc.dram_tensor("out", [O, NLOC], f32, kind="ExternalOutput")

    with ExitStack() as ctx:
        tc = ctx.enter_context(tile.TileContext(nc))
        sb = ctx.enter_context(tc.tile_pool(name="sb", bufs=1))
        sbx = ctx.enter_context(tc.tile_pool(name="sbx", bufs=3))
        sbw = ctx.enter_context(tc.tile_pool(name="sbw", bufs=2))
        sbm = ctx.enter_context(tc.tile_pool(name="sbm", bufs=2))
        sbo = ctx.enter_context(tc.tile_pool(name="sbo", bufs=3))
        sbi = ctx.enter_context(tc.tile_pool(name="sbi", bufs=4))
        psS = ctx.enter_context(tc.tile_pool(name="psS", bufs=3, space="PSUM"))
        psB = ctx.enter_context(tc.tile_pool(name="psB", bufs=1, space="PSUM"))
        psT = ctx.enter_context(tc.tile_pool(name="psT", bufs=1, space="PSUM"))

        # ---------- persistent SBUF ----------
        bdh_sb = sb.tile([128, G, 128], f16)
        bdl_sb = sb.tile([128, G, 128], f16)
        nc2hl_sb = sb.tile([2, 1024], f16)
        or2_sb = sb.tile([2, 128], f16)
        biasT2_sb = sb.tile([128, OTILES], f32)
        inv_sb = sb.tile([128, 1], f32)
        scale_sb = sb.tile([128, 1], f32)
        kiota_sb = sb.tile([128, 1], f16)
        ioneg_sb = sb.tile([128, 1024], f16)
        idb_sb = sb.tile([128, 128], f16)
        q_sb = sb.tile([128, G, O], f16)
        oh_sb = sb.tile([128, G, NLOC], f16)
        idxT_sb = sb.tile([64, NLOC], f16)
        c1536_sb = sb.tile([128, 1], f32)

        # ---------- const + input DMAs ----------
        nc.gpsimd.dma_start(bdh_sb[:], bdh_d[:])
        nc.gpsimd.dma_start(bdl_sb[:], bdl_d[:])
        nc.gpsimd.dma_start(nc2hl_sb[:], nc2hl_d[:])
        nc.gpsimd.dma_start(or2_sb[:], or2_d[:])
        nc.gpsimd.dma_start(inv_sb[:], inv_d[:])
        nc.gpsimd.dma_start(scale_sb[:], scl_d[:])
        nc.gpsimd.dma_start(kiota_sb[:], kiota_d[:])
        nc.gpsimd.dma_start(ioneg_sb[:], ioneg_d[:])
        nc.gpsimd.dma_start(idb_sb[:], idb_d[:])
        nc.gpsimd.dma_start(biasT2_sb[:], biasT2_d[:])
        nc.vector.memset(c1536_sb[:], OFF)

        # x token tiles (hi/lo fp16) on the scalar engine's DMA queue;
        # xh tiles stream first (hi passes run while xl still in flight)
        xh_tiles = [
            sbx.tile([128, G, 128], f16, tag="xh", name=f"xh{t}")
            for t in range(TT)
        ]
        xl_tiles = [
            sbx.tile([128, G, 128], f16, tag="xl", name=f"xl{t}")
            for t in range(TT)
        ]
        dma_order = [("h", t) for t in range(4)]
        for t in range(4):
            dma_order += [("l", t), ("h", t + 4)]
        dma_order += [("l", t) for t in range(4, TT)]
        for kind, t in dma_order:
            tile_ = xh_tiles[t] if kind == "h" else xl_tiles[t]
            src_ = xh_d if kind == "h" else xl_d
            nc.scalar.dma_start(tile_[:], src_[:, t])
        # w chunks on the sync engine's DMA queue
        w_tiles = []
        for c in range(NCH):
            w_t = sbw.tile([128, G, 512], f16, tag="w16", name=f"w16c{c}")
            for p in range(4):
                nc.sync.dma_start(
                    w_t[:, 2 * p : 2 * p + 2, :], w16_d[:, c, 2 * p : 2 * p + 2]
                )
            w_tiles.append(w_t)

        # ------ phase L: lut pair (2 groups x 512 o-cols) + fused quantize -
        # two matmuls into one [128,1024] psS tile; ONE scalar op drains the
        # PSUM: u = round(lut*127/max) + 1536 via the fp16-ulp RNE trick
        def emit_lut_pair(c, p):
            g = 2 * p
            lp = psS.tile([128, 1024], f32, tag="sc", name=f"lp{c}_{p}")
            for i in range(2):
                nc.tensor.matmul(
                    lp[:, i * 512 : (i + 1) * 512], bdh_sb[:, g + i, :],
                    w_tiles[c][:, g + i, :],
                    start=True, stop=True, skip_group_check=True,
                )
            nc.scalar.activation(
                q_sb[:, g : g + 2, c * 512 : (c + 1) * 512],
                lp[:].rearrange("q (a b) -> q a b", b=512),
                AF.Identity, bias=c1536_sb[:, 0:1], scale=inv_sb[:, 0:1],
            )

        # ---------- phase S: scores -> first-max index encoding ----------
        idxt_tiles = {}
        sc_tiles = {}

        # hi passes need only xh (arrives first): init + xh.bh + xh.bl
        def emit_score_hi(t):
            sc_ps = psS.tile([128, 1024], f32, tag="sc", name=f"sc{t}")
            sc_tiles[t] = sc_ps
            for h in range(2):
                nc.tensor.matmul(
                    sc_ps[:, h * 512 : (h + 1) * 512], or2_sb[:],
                    nc2hl_sb[:, h * 512 : (h + 1) * 512],
                    start=True, stop=False, skip_group_check=True,
                )
            for g in range(G):
                nc.tensor.matmul(
                    sc_ps[:, g * 128 : (g + 1) * 128],
                    xh_tiles[t][:, g, :], bdh_sb[:, g, :],
                    start=False, stop=False, skip_group_check=True,
                )
                nc.tensor.matmul(
                    sc_ps[:, g * 128 : (g + 1) * 128],
                    xh_tiles[t][:, g, :], bdl_sb[:, g, :],
                    start=False, stop=False, skip_group_check=True,
                )

        # lo pass (xl.bh) + the argmax chain on DVE
        def emit_score_lo(t):
            sc_ps = sc_tiles[t]
            for g in range(G):
                nc.tensor.matmul(
                    sc_ps[:, g * 128 : (g + 1) * 128],
                    xl_tiles[t][:, g, :], bdh_sb[:, g, :],
                    start=False, stop=(g % 4 == 3), skip_group_check=True,
                )
            maxb = sbm.tile([128, C], f32, tag="maxb", name=f"maxb{t}")
            nc.vector.tensor_reduce(
                maxb[:], sc_ps[:].rearrange("p (c k) -> p c k", k=KC),
                axis=X, op=AO.max,
            )
            mask = sbm.tile([128, 1024], f16, tag="mask", name=f"mask{t}")
            nc.vector.tensor_tensor(
                mask[:].rearrange("p (c k) -> p c k", k=KC),
                sc_ps[:].rearrange("p (c k) -> p c k", k=KC),
                maxb[:].rearrange("p (c u) -> p c u", u=1).broadcast_to((128, C, KC)),
                op=AO.is_equal,
            )
            # iv = mask*64 + (15-k): max picks the first (smallest-k) hit
            nc.vector.scalar_tensor_tensor(
                mask[:], mask[:], 64.0, ioneg_sb[:], op0=AO.mult, op1=AO.add
            )
            idxt = sbi.tile([128, C], f16, tag="idxt", name=f"idxt{t}")
            nc.vector.tensor_reduce(
                idxt[:], mask[:].rearrange("p (c k) -> p c k", k=KC),
                axis=X, op=AO.max,
            )
            idxt_tiles[t] = idxt

        # deferred: transpose tile t's index row into idxT (PE + DVE copy);
        # emitted a few slots after the chain so the PE never waits on it
        def emit_tp(t):
            tok = slice(t * 128, (t + 1) * 128)
            tp_ps = psT.tile([64, 128], f16, tag="tp", name=f"tp{t}")
            nc.tensor.transpose(tp_ps[:], idxt_tiles[t][:], idb_sb[:])
            nc.vector.tensor_copy(idxT_sb[:, tok], tp_ps[:])

        # one-hot expansion for (group g, token half h)
        def emit_oh(g, h):
            cols = slice(h * 512, (h + 1) * 512)
            idxb = sbi.tile([128, 512], f16, tag="idxb", name=f"idxb{g}_{h}")
            nc.gpsimd.dma_start(
                idxb[:],
                idxT_sb[g * 8 : (g + 1) * 8, cols]
                .rearrange("j (n u) -> j u n", u=1)
                .broadcast_to((8, KC, 512)),
            )
            nc.vector.tensor_tensor(
                oh_sb[:, g, cols], idxb[:],
                kiota_sb[:, 0:1].broadcast_to((128, 512)),
                op=AO.is_equal,
            )

        # ---------- phase G: gather matmuls + epilogue ----------
        def emit_gather(ot, h0, h1, eng):
            cols = slice(h0 * 512, h1 * 512)
            ncol = (h1 - h0) * 512
            pool = psB if ncol == 512 else psS
            gat = pool.tile(
                [128, ncol], f32, tag="gb" if ncol == 512 else "sc",
                name=f"gat{ot}_{h0}{h1}",
            )
            for g in range(G):
                for hh in range(h0, h1):
                    nc.tensor.matmul(
                        gat[:, (hh - h0) * 512 : (hh - h0 + 1) * 512],
                        q_sb[:, g, ot * 128 : (ot + 1) * 128],
                        oh_sb[:, g, hh * 512 : (hh + 1) * 512],
                        start=(g == 0), stop=(g == G - 1),
                        skip_group_check=True,
                    )
            o_sb = sbo.tile(
                [128, ncol], f32, tag="osb" if ncol == 512 else "osbF",
                name=f"osb{ot}_{h0}",
            )
            if eng == "s":
                nc.scalar.activation(
                    o_sb[:], gat[:], AF.Identity,
                    bias=biasT2_sb[:, ot : ot + 1], scale=scale_sb[:, 0:1],
                )
            else:
                nc.vector.scalar_tensor_tensor(
                    o_sb[:], gat[:], scale_sb[:, 0:1],
                    biasT2_sb[:, ot : ot + 1].broadcast_to((128, ncol)),
                    op0=AO.mult, op1=AO.add,
                )
            nc.sync.dma_start(out_d[ot * 128 : (ot + 1) * 128, cols], o_sb[:])

        # ---------- interleaved emission (PE queue is in-order!) ----------
        # lut pairs + score tiles first come data-ready; G h0 o-tiles slot in
        # as their q chunk (scalar-drain paced) and oh h0 become available
        def L(c):
            for p in range(4):
                emit_lut_pair(c, p)

        def Lp(c, p):
            emit_lut_pair(c, p)

        emit_score_hi(0); emit_score_hi(1)
        Lp(0, 0)
        emit_score_lo(0)
        Lp(0, 1)
        emit_score_hi(2)
        Lp(0, 2)
        emit_score_lo(1)
        Lp(0, 3)
        emit_tp(0)
        emit_score_hi(3)
        Lp(1, 0)
        emit_score_lo(2)
        Lp(1, 1)
        emit_tp(1)
        emit_score_hi(4)
        Lp(1, 2)
        emit_score_lo(3)
        Lp(1, 3)
        emit_tp(2)
        emit_score_hi(5)
        Lp(2, 0)
        emit_score_lo(4)
        Lp(2, 1)
        emit_tp(3)
        for g in range(G):
            emit_oh(g, 0)
        emit_score_hi(6)
        Lp(2, 2)
        emit_score_lo(5)
        Lp(2, 3)
        emit_tp(4)
        emit_score_hi(7)
        Lp(3, 0)
        emit_score_lo(6)
        Lp(3, 1)
        emit_tp(5)
        Lp(3, 2)
        emit_score_lo(7)
        Lp(3, 3)
        emit_tp(6)
        emit_tp(7)
        for g in range(G):
            emit_oh(g, 1)
        emit_gather(0, 0, 1, "s"); Lp(4, 0); Lp(4, 1)
        emit_gather(1, 0, 1, "s"); Lp(4, 2); Lp(4, 3)
        emit_gather(2, 0, 1, "s"); Lp(5, 0); Lp(5, 1)
        emit_gather(3, 0, 1, "s"); Lp(5, 2); Lp(5, 3)
        emit_gather(4, 0, 1, "s"); Lp(6, 0); Lp(6, 1)
        emit_gather(5, 0, 1, "s"); Lp(6, 2); Lp(6, 3)
        emit_gather(6, 0, 1, "s"); Lp(7, 0); Lp(7, 1)
        emit_gather(7, 0, 1, "s"); Lp(7, 2); Lp(7, 3)
        for ot in range(8):
            emit_gather(ot, 1, 2, "s" if ot % 2 else "v")
        for ot in range(8, OTILES):
            emit_gather(ot, 0, 2, "s" if ot % 2 else "v")

    nc.compile()
    return nc


def _consts():
    kiota = (79.0 - np.arange(128, dtype=np.float32) % KC).reshape(128, 1).astype(np.float16)
    ioneg = np.tile(
        15.0 - (np.arange(1024, dtype=np.float32) % KC), (128, 1)
    ).astype(np.float16)
    idb = np.eye(128, dtype=np.float16)
    return kiota, ioneg, idb


def _prep_inputs(x, centroids, weight, bias):
    """Host-side shard/layout prep + the weight-derived quant scale."""
    kiota, ioneg, idb = _consts()
    # block-diagonal centroids^T: bd[16j+s, g, 16j+k] = centroids[8g+j, k, s]
    bd = np.zeros((128, G, 128), np.float32)
    for g in range(G):
        for j in range(8):
            bd[16 * j : 16 * (j + 1), g, 16 * j : 16 * (j + 1)] = centroids[
                8 * g + j
            ].T
    bdh = bd.astype(np.float16)
    bdl = (bd - bdh.astype(np.float32)).astype(np.float16)
    w16 = np.ascontiguousarray(
        weight.reshape(G, 128, NCH, 512).transpose(1, 2, 0, 3)
    ).astype(np.float16)
    nc2 = (-0.5 * (centroids.astype(np.float64) ** 2).sum(-1)).astype(
        np.float32
    ).reshape(1, C * KC)
    nc2h = nc2.astype(np.float16)
    nc2l = (nc2 - nc2h.astype(np.float32)).astype(np.float16)
    nc2hl = np.concatenate([nc2h, nc2l], axis=0)
    or2 = np.ones((2, 128), np.float16)
    # weight-derived int8 quant scale (x-independent; offline in real AMM)
    lut = np.einsum(
        "cks,cso->cko", centroids.astype(np.float32),
        weight.astype(np.float32),
    )
    amax = np.float64(np.abs(lut).max())
    scale = np.float32(amax / 127.0)
    inv = np.full((128, 1), np.float32(127.0 / amax), np.float32)
    scl = np.full((128, 1), scale, np.float32)
    biasT2 = np.ascontiguousarray(
        bias.reshape(OTILES, 128).T - OFFSUM * scale
    ).astype(np.float32)
    common = dict(
        w16=w16, bdh=bdh, bdl=bdl, nc2hl=nc2hl, or2=or2,
        biasT2=biasT2, inv=inv, scl=scl, kiota=kiota, ioneg=ioneg, idb=idb,
    )
    in_maps = []
    for i in range(NCORES):
        xs = x[i * NLOC : (i + 1) * NLOC, :]  # (1024, 1024)
        xt = np.ascontiguousarray(
            xs.T.reshape(G, 128, TT, 128).transpose(1, 2, 0, 3)
        )  # [p, t, g, n]
        xh = xt.astype(np.float16)
        xl = (xt - xh.astype(np.float32)).astype(np.float16)
        m = dict(common)
        m.update(xh=xh, xl=xl)
        in_maps.append(m)
    return in_maps


def kernel(x, centroids, weight, inverse_temperature_logit, bias, **_):
    from concourse.bass_utils import run_bass_kernel_spmd

    x = np.asarray(x, np.float32)
    centroids = np.asarray(centroids, np.float32)
    weight = np.asarray(weight, np.float32)
    bias = np.asarray(bias, np.float32)

    if "nc" not in _CACHED:
        _CACHED["nc"] = build_nc()
    nc = _CACHED["nc"]

    in_maps = _prep_inputs(x, centroids, weight, bias)
    res = run_bass_kernel_spmd(nc, in_maps, core_ids=list(range(NCORES)))
    out = np.empty((N_TOKENS, O), np.float32)
    for i in range(NCORES):
        out[i * NLOC : (i + 1) * NLOC, :] = res.results[i]["out"].T
    return out
